# revision 18
# baseline (speedup 1.0000x reference)
"""MoE FFN (top-2 of 8 experts) on 8 TRN2 NeuronCores, expert-parallel.

Strategy (per spec sharding_hint): shard experts 1:1 across the 8 cores.
The router (gate matmul + top-k + softmax + aux loss, ~0.03% of FLOPs) runs
on host as part of the dispatch/combine sharding step; each core runs the
two expert FFN matmuls (99.97% of FLOPs) for the tokens routed to its
expert, in fp32r (tf32-like) precision on the PE array.

Device layout per core (expert e):
  xT  [D, C]   fp32r  tokens routed to e, transposed, zero-padded to C
  w1  [D, FF]  fp32r
  w2  [FF, D]  fp32r
  b1  [128, FF/128] fp32   (b1.reshape(FF/128,128).T  per-partition bias)
  b2  [128, D/128]  fp32
  yT  [D, C]   fp32   output (transposed)

Software pipeline over NT token tiles: phase-1 (16 h-blocks: 8-step accum
over D + fused bias/GELU on ACT -> fp32r SBUF, h double-buffered) and
phase-2 (8 y-blocks: 16-step accum over FF + fused bias, DMA out),
interleaved P1(0) P1(1) P2(0) P1(2) ... so the w2 weight stream (2/3 of
all DMA bytes) hides under early compute.
"""
import os
import sys

sys.path.insert(0, '/opt/trn_rl_repo')

import numpy as np

import concourse.bacc as bacc
import concourse.mybir as mybir
from concourse.tile import TileContext
from concourse.bass_utils import run_bass_kernel_spmd

B, T, D, E, TOPK, FF = 2, 2048, 1024, 8, 2, 2048
N_CORES = 8
KD = D // 128     # 8  k-tiles for first matmul
MF = FF // 128    # 16 h blocks
KF = FF // 128    # 16 k-tiles for second matmul
MD = D // 128     # 8  y blocks

f32 = mybir.dt.float32
f32r = mybir.dt.float32r
GELU = mybir.ActivationFunctionType.Gelu
IDENT = mybir.ActivationFunctionType.Identity

_built = {}


def _plan_tiles(max_count):
    """Pick capacity C and token-tile sizes: near-equal tiles, each in
    [256, 512] (fp32r runs 4x slower below a 256-wide moving dim)."""
    c0 = max(int(max_count), 256)
    n = max(1, -(-c0 // 512))
    while c0 / n < 256 and n > 1:
        n -= 1
    # prefer >=4 tiles when allowed: smaller tiles pipeline better
    while n < 4 and (c0 + n) // (n + 1) >= 256:
        n += 1
    tile = -(-c0 // n)
    tile = -(-tile // 4) * 4          # multiple of 4 columns
    return tile * n, [tile] * n


def _build(C, tiles, repeat=1, barrier=False):
    key = (C, tuple(tiles), repeat, barrier)
    if key in _built:
        return _built[key]
    NT = len(tiles)
    nc = bacc.Bacc("TRN2", target_bir_lowering=False, debug=False,
                   num_devices=N_CORES)
    xT = nc.dram_tensor("xT", [D, C], f32r, kind="ExternalInput")
    w1 = nc.dram_tensor("w1", [D, FF], f32r, kind="ExternalInput")
    w2 = nc.dram_tensor("w2", [FF, D], f32r, kind="ExternalInput")
    b1 = nc.dram_tensor("b1", [128, MF], f32, kind="ExternalInput")
    b2 = nc.dram_tensor("b2", [128, MD], f32, kind="ExternalInput")
    yT = nc.dram_tensor("yT", [D, C], f32, kind="ExternalOutput")

    starts = np.concatenate([[0], np.cumsum(tiles)]).tolist()

    # DRAM views with the 128-partition block factored out:
    # row (k*128 + p) -> [p, k, cols]
    w1v = w1[:, :].rearrange("(k p) f -> p k f", p=128)
    w2v = w2[:, :].rearrange("(k p) d -> p k d", p=128)
    xv = xT[:, :].rearrange("(k p) c -> p k c", p=128)
    yv = yT[:, :].rearrange("(m p) c -> p m c", p=128)

    with TileContext(nc) as tc:
        with tc.tile_pool(name="wpool", bufs=1) as wpool, \
             tc.tile_pool(name="xpool", bufs=2) as xpool, \
             tc.tile_pool(name="hpool", bufs=3) as hpool, \
             tc.tile_pool(name="ypool", bufs=1) as ypool, \
             tc.tile_pool(name="cpool", bufs=1) as cpool, \
             tc.tile_pool(name="ph_pool", bufs=3, space="PSUM") as ph_pool, \
             tc.tile_pool(name="py_pool", bufs=3, space="PSUM") as py_pool:
            b1t = cpool.tile_from(b1[:, :], name="b1t")
            b2t = cpool.tile_from(b2[:, :], name="b2t")
            hs = [None] * NT
            w1t = w2t = None

            def load_w1():
                nonlocal w1t, w2t
                w1t = wpool.tile([128, KD, FF], f32r, name="w1t")
                w2t = wpool.tile([128, KF, D], f32r, name="w2t")
                for j in range(0, KD, 2):
                    nc.sync.dma_start(out=w1t[:, j:j + 2, :],
                                      in_=w1v[:, j:j + 2, :])

            def load_x(t):
                ts = slice(starts[t], starts[t + 1])
                xt = xpool.tile([128, KD, tiles[t]], f32r, name="xt")
                nc.sync.dma_start(out=xt[:, :, :], in_=xv[:, :, ts])
                return xt

            def load_w2(lo, hi):
                for j in range(lo, hi, 2):
                    nc.sync.dma_start(out=w2t[:, j:j + 2, :],
                                      in_=w2v[:, j:j + 2, :])

            def phase1(t, xt):
                TN = tiles[t]
                hm = hpool.tile([128, MF, TN], f32r, name="hm")
                for mf in range(MF):
                    ph = ph_pool.tile([128, TN], f32, name="ph")
                    for kd in range(KD):
                        nc.tensor.matmul(
                            ph,
                            lhsT=w1t[:, kd, mf * 128:(mf + 1) * 128],
                            rhs=xt[:, kd, :],
                            start=(kd == 0), stop=(kd == KD - 1))
                    nc.scalar.activation(hm[:, mf, :], ph, GELU,
                                         bias=b1t[:, mf:mf + 1])
                hs[t] = hm

            def phase2(t):
                TN = tiles[t]
                ts = slice(starts[t], starts[t + 1])
                yt = ypool.tile([128, MD, TN], f32, name="yt")
                for md in range(MD):
                    py = py_pool.tile([128, TN], f32, name="py")
                    for kf in range(KF):
                        nc.tensor.matmul(
                            py,
                            lhsT=w2t[:, kf, md * 128:(md + 1) * 128],
                            rhs=hs[t][:, kf, :],
                            start=(kf == 0), stop=(kf == KF - 1))
                    nc.scalar.activation(yt[:, md, :], py, IDENT,
                                         bias=b2t[:, md:md + 1])
                nc.sync.dma_start(out=yv[:, :, ts], in_=yt[:, :, :])
                hs[t] = None

            for _rep in range(repeat):
                load_w1()
                if NT == 1:
                    xt0 = load_x(0)
                    load_w2(0, KF)
                    phase1(0, xt0)
                    phase2(0)
                elif NT == 2:
                    xt0 = load_x(0)
                    xt1 = load_x(1)
                    phase1(0, xt0)
                    load_w2(0, KF // 2)
                    phase1(1, xt1)
                    load_w2(KF // 2, KF)
                    phase2(0)
                    phase2(1)
                else:
                    # phase-1-heavy prologue: 3 h-sets in flight so the w2
                    # stream (2/3 of DMA bytes) hides under compute
                    xt0 = load_x(0)
                    xt1 = load_x(1)
                    phase1(0, xt0)
                    load_w2(0, KF // 2)
                    phase1(1, xt1)
                    load_w2(KF // 2, KF)
                    phase1(2, load_x(2))
                    phase2(0)
                    for t in range(3, NT):
                        phase1(t, load_x(t))
                        phase2(t - 2)
                    phase2(NT - 2)
                    phase2(NT - 1)
                if barrier and _rep != repeat - 1:
                    tc.strict_bb_all_engine_barrier()
    nc.finalize()
    _built[key] = nc
    return nc


def kernel(x, gate_w, w1, w2, b1, b2):
    x = np.asarray(x, dtype=np.float32)
    gate_w = np.asarray(gate_w, dtype=np.float32)
    w1 = np.asarray(w1, dtype=np.float32)
    w2 = np.asarray(w2, dtype=np.float32)
    b1 = np.asarray(b1, dtype=np.float32)
    b2 = np.asarray(b2, dtype=np.float32)

    NTOK = B * T
    x2d = x.reshape(NTOK, D)

    # ---- host router (replicated-gate sharding step) ----
    logits = x2d @ gate_w.T                         # [NTOK, E] fp32
    order = np.argsort(-logits, axis=1, kind="stable")
    top_idx = order[:, :TOPK]                       # [NTOK, K] matches top_k
    top_logits = np.take_along_axis(logits, top_idx, axis=1)
    tl64 = top_logits.astype(np.float64)
    wexp = np.exp(tl64 - tl64.max(axis=1, keepdims=True))
    weights = (wexp / wexp.sum(axis=1, keepdims=True)).astype(np.float32)

    l64 = logits.astype(np.float64)
    p = np.exp(l64 - l64.max(axis=1, keepdims=True))
    probs = p / p.sum(axis=1, keepdims=True)        # [NTOK, E]
    frac_probs = probs.mean(axis=0)
    frac_tokens = np.bincount(top_idx[:, 0], minlength=E) / NTOK
    aux = E * np.sum(frac_tokens * frac_probs) + np.mean(l64 ** 2) * 0.001
    aux_loss = np.float32(aux)

    # ---- dispatch: group tokens by expert ----
    flat_tok = np.repeat(np.arange(NTOK), TOPK)
    flat_e = top_idx.ravel()
    flat_w = weights.ravel()
    esort = np.argsort(flat_e, kind="stable")
    s_tok, s_e, s_w = flat_tok[esort], flat_e[esort], flat_w[esort]
    bounds = np.searchsorted(s_e, np.arange(E + 1))
    counts = np.diff(bounds)
    C, tiles = _plan_tiles(counts.max())

    tok_ids, tok_w, in_maps = [], [], []
    for e in range(E):
        ids = s_tok[bounds[e]:bounds[e + 1]]
        wts = s_w[bounds[e]:bounds[e + 1]]
        ce = len(ids)
        tok_ids.append(ids)
        tok_w.append(wts)
        xTe = np.zeros((D, C), dtype=np.float32)
        xTe[:, :ce] = x2d[ids].T
        in_maps.append({
            "xT": xTe,
            "w1": np.ascontiguousarray(w1[e]),
            "w2": np.ascontiguousarray(w2[e]),
            "b1": np.ascontiguousarray(b1[e].reshape(MF, 128).T),
            "b2": np.ascontiguousarray(b2[e].reshape(MD, 128).T),
        })

    # ---- device: expert FFN on 8 cores ----
    nc = _build(C, tiles)
    # NTFF tracing needs antenv.axon_hooks, absent in this container —
    # make sure an inherited BASS_TRACE can't crash the run.
    os.environ["BASS_NEVER_TRACE"] = "1"
    res = run_bass_kernel_spmd(nc, in_maps, list(range(N_CORES)))
    kernel.last_results = res

    # ---- combine ----
    out2d = np.zeros((NTOK, D), dtype=np.float32)
    for e in range(E):
        ce = len(tok_ids[e])
        ye = res.results[e]["yT"][:, :ce]           # [D, ce]
        # token ids are unique within one expert (top-2 experts distinct)
        out2d[tok_ids[e]] += (tok_w[e][None, :] * ye).T
    out = out2d.reshape(B, T, D)
    return out, aux_loss


if __name__ == "__main__":
    data = np.load("/root/problem/inputs_cache.npz")
    inputs = {k: data[k] for k in data.files}
    ref = np.load("/root/problem/ref_cache.npz")
    out, aux = kernel(**inputs)
    ref_out, ref_aux = ref["out"], ref["aux"]
    denom = np.abs(ref_out).max()
    err = np.abs(out - ref_out).max()
    print(f"out absmax err: {err:.3e}  rel: {err/denom:.3e}")
    print(f"aux: {aux!r} vs ref {ref_aux!r}  diff {abs(float(aux)-float(ref_aux)):.3e}")


# revision 25
# speedup vs baseline: 1.0014x; 1.0014x over previous
"""MoE FFN (top-2 of 8 experts) on 8 TRN2 NeuronCores, expert-parallel.

Strategy (per spec sharding_hint): shard experts 1:1 across the 8 cores.
The router (gate matmul + top-k + softmax + aux loss, ~0.03% of FLOPs) runs
on host as part of the dispatch/combine sharding step; each core runs the
two expert FFN matmuls (99.97% of FLOPs) for the tokens routed to its
expert, in fp32r (tf32-like) precision on the PE array.

Device layout per core (expert e):
  xT  [D, C]   fp32r  tokens routed to e, transposed, zero-padded to C
  w1  [D, FF]  fp32r
  w2  [FF, D]  fp32r
  b1  [128, FF/128] fp32   (b1.reshape(FF/128,128).T  per-partition bias)
  b2  [128, D/128]  fp32
  yT  [D, C]   fp32   output (transposed)

Software pipeline over NT token tiles: phase-1 (16 h-blocks: 8-step accum
over D + fused bias/GELU on ACT -> fp32r SBUF, h double-buffered) and
phase-2 (8 y-blocks: 16-step accum over FF + fused bias, DMA out),
interleaved P1(0) P1(1) P2(0) P1(2) ... so the w2 weight stream (2/3 of
all DMA bytes) hides under early compute.
"""
import os
import sys

sys.path.insert(0, '/opt/trn_rl_repo')

import ml_dtypes
import numpy as np

import concourse.bacc as bacc
import concourse.mybir as mybir
from concourse.tile import TileContext
from concourse.bass_utils import run_bass_kernel_spmd

B, T, D, E, TOPK, FF = 2, 2048, 1024, 8, 2, 2048
N_CORES = 8
KD = D // 128     # 8  k-tiles for first matmul
MF = FF // 128    # 16 h blocks
KF = FF // 128    # 16 k-tiles for second matmul
MD = D // 128     # 8  y blocks

f32 = mybir.dt.float32
f32r = mybir.dt.float32r
bf16 = mybir.dt.bfloat16
# "f32r": everything tf32-like (rel err ~2e-4).  "mixed": second layer
# (h, w2) in bf16 (rel err ~2e-3) — halves the w2 weight stream, the
# single-shot critical path.
MODE = "f32r"
GELU = mybir.ActivationFunctionType.Gelu
IDENT = mybir.ActivationFunctionType.Identity

_built = {}


def _plan_tiles(max_count):
    """Pick capacity C and token-tile sizes: near-equal tiles, each in
    [256, 512] (fp32r runs 4x slower below a 256-wide moving dim)."""
    c0 = max(int(max_count), 256)
    n = max(1, -(-c0 // 512))
    while c0 / n < 256 and n > 1:
        n -= 1
    # prefer >=4 tiles when allowed: smaller tiles pipeline better
    while n < 4 and (c0 + n) // (n + 1) >= 256:
        n += 1
    tile = -(-c0 // n)
    tile = -(-tile // 4) * 4          # multiple of 4 columns
    return tile * n, [tile] * n


def _build(C, tiles, repeat=1, barrier=False, mode=None):
    mode = mode or MODE
    w2dt = bf16 if mode in ("mixed", "bf16") else f32r
    w1dt = bf16 if mode == "bf16" else f32r
    key = (C, tuple(tiles), repeat, barrier, mode)
    if key in _built:
        return _built[key]
    NT = len(tiles)
    nc = bacc.Bacc("TRN2", target_bir_lowering=False, debug=False,
                   num_devices=N_CORES)
    xT = nc.dram_tensor("xT", [D, C], w1dt, kind="ExternalInput")
    w1 = nc.dram_tensor("w1", [D, FF], w1dt, kind="ExternalInput")
    w2 = nc.dram_tensor("w2", [FF, D], w2dt, kind="ExternalInput")
    b1 = nc.dram_tensor("b1", [128, MF], f32, kind="ExternalInput")
    b2 = nc.dram_tensor("b2", [128, MD], f32, kind="ExternalInput")
    yT = nc.dram_tensor("yT", [D, C], f32, kind="ExternalOutput")

    starts = np.concatenate([[0], np.cumsum(tiles)]).tolist()

    # DRAM views with the 128-partition block factored out:
    # row (k*128 + p) -> [p, k, cols]
    w1v = w1[:, :].rearrange("(k p) f -> p k f", p=128)
    w2v = w2[:, :].rearrange("(k p) d -> p k d", p=128)
    xv = xT[:, :].rearrange("(k p) c -> p k c", p=128)
    yv = yT[:, :].rearrange("(m p) c -> p m c", p=128)

    with TileContext(nc) as tc:
        with tc.tile_pool(name="wpool", bufs=1) as wpool, \
             tc.tile_pool(name="xpool", bufs=2) as xpool, \
             tc.tile_pool(name="hpool", bufs=(4 if mode in ("mixed", "bf16") else 3)) as hpool, \
             tc.tile_pool(name="ypool", bufs=1) as ypool, \
             tc.tile_pool(name="cpool", bufs=1) as cpool, \
             tc.tile_pool(name="ph_pool", bufs=3, space="PSUM") as ph_pool, \
             tc.tile_pool(name="py_pool", bufs=3, space="PSUM") as py_pool:
            b1t = cpool.tile_from(b1[:, :], name="b1t")
            b2t = cpool.tile_from(b2[:, :], name="b2t")
            hs = [None] * NT
            w1t = w2t = None

            def load_w1():
                nonlocal w1t, w2t
                w1t = wpool.tile([128, KD, FF], w1dt, name="w1t")
                w2t = wpool.tile([128, KF, D], w2dt, name="w2t")
                for j in range(0, KD, 2):
                    nc.sync.dma_start(out=w1t[:, j:j + 2, :],
                                      in_=w1v[:, j:j + 2, :])

            def load_x(t):
                ts = slice(starts[t], starts[t + 1])
                xt = xpool.tile([128, KD, tiles[t]], w1dt, name="xt")
                nc.sync.dma_start(out=xt[:, :, :], in_=xv[:, :, ts])
                return xt

            def load_w2(lo, hi):
                for j in range(lo, hi, 2):
                    nc.sync.dma_start(out=w2t[:, j:j + 2, :],
                                      in_=w2v[:, j:j + 2, :])

            def phase1(t, xt):
                TN = tiles[t]
                hm = hpool.tile([128, MF, TN], w2dt, name="hm")
                for mf in range(MF):
                    ph = ph_pool.tile([128, TN], f32, name="ph")
                    for kd in range(KD):
                        nc.tensor.matmul(
                            ph,
                            lhsT=w1t[:, kd, mf * 128:(mf + 1) * 128],
                            rhs=xt[:, kd, :],
                            start=(kd == 0), stop=(kd == KD - 1))
                    nc.scalar.activation(hm[:, mf, :], ph, GELU,
                                         bias=b1t[:, mf:mf + 1])
                hs[t] = hm

            def phase2(t):
                TN = tiles[t]
                ts = slice(starts[t], starts[t + 1])
                yt = ypool.tile([128, MD, TN], f32, name="yt")
                for md in range(MD):
                    py = py_pool.tile([128, TN], f32, name="py")
                    for kf in range(KF):
                        nc.tensor.matmul(
                            py,
                            lhsT=w2t[:, kf, md * 128:(md + 1) * 128],
                            rhs=hs[t][:, kf, :],
                            start=(kf == 0), stop=(kf == KF - 1))
                    nc.scalar.activation(yt[:, md, :], py, IDENT,
                                         bias=b2t[:, md:md + 1])
                nc.sync.dma_start(out=yv[:, :, ts], in_=yt[:, :, :])
                hs[t] = None

            for _rep in range(repeat):
                load_w1()
                if NT == 1:
                    xt0 = load_x(0)
                    load_w2(0, KF)
                    phase1(0, xt0)
                    phase2(0)
                elif NT == 2:
                    xt0 = load_x(0)
                    xt1 = load_x(1)
                    phase1(0, xt0)
                    load_w2(0, KF // 2)
                    phase1(1, xt1)
                    load_w2(KF // 2, KF)
                    phase2(0)
                    phase2(1)
                elif mode in ("mixed", "bf16") and NT <= 4:
                    # h is NT-buffered: run every phase-1 before any
                    # phase-2 so the (halved) w2 stream fully hides
                    xts = [load_x(0), load_x(1)]
                    phase1(0, xts[0])
                    load_w2(0, KF // 2)
                    xts.append(load_x(2))
                    phase1(1, xts[1])
                    load_w2(KF // 2, KF)
                    if NT > 3:
                        xts.append(load_x(3))
                    for t in range(2, NT):
                        phase1(t, xts[t])
                    for t in range(NT):
                        phase2(t)
                else:
                    # phase-1-heavy prologue: 3 h-sets in flight so the w2
                    # stream (2/3 of DMA bytes) hides under compute.  The
                    # small x loads are traced BEFORE each w2 chunk so they
                    # never queue behind the 16.8MB weight stream (x tiles
                    # share 2 slots, so x(t) DMA self-serializes on the
                    # release of x(t-2) via its WAR dependency).
                    xt0 = load_x(0)
                    xt1 = load_x(1)
                    phase1(0, xt0)
                    xts = {2: load_x(2)} if NT > 2 else {}
                    load_w2(0, KF // 2)
                    phase1(1, xt1)
                    if NT > 3:
                        xts[3] = load_x(3)
                    load_w2(KF // 2, KF)
                    phase1(2, xts[2])
                    phase2(0)
                    for t in range(3, NT):
                        xts[t] = xts.get(t) or load_x(t)
                        phase1(t, xts[t])
                        phase2(t - 2)
                    phase2(NT - 2)
                    phase2(NT - 1)
                if barrier and _rep != repeat - 1:
                    tc.strict_bb_all_engine_barrier()
    nc.finalize()
    _built[key] = nc
    return nc


def kernel(x, gate_w, w1, w2, b1, b2):
    x = np.asarray(x, dtype=np.float32)
    gate_w = np.asarray(gate_w, dtype=np.float32)
    w1 = np.asarray(w1, dtype=np.float32)
    w2 = np.asarray(w2, dtype=np.float32)
    b1 = np.asarray(b1, dtype=np.float32)
    b2 = np.asarray(b2, dtype=np.float32)

    NTOK = B * T
    x2d = x.reshape(NTOK, D)

    # ---- host router (replicated-gate sharding step) ----
    logits = x2d @ gate_w.T                         # [NTOK, E] fp32
    order = np.argsort(-logits, axis=1, kind="stable")
    top_idx = order[:, :TOPK]                       # [NTOK, K] matches top_k
    top_logits = np.take_along_axis(logits, top_idx, axis=1)
    tl64 = top_logits.astype(np.float64)
    wexp = np.exp(tl64 - tl64.max(axis=1, keepdims=True))
    weights = (wexp / wexp.sum(axis=1, keepdims=True)).astype(np.float32)

    l64 = logits.astype(np.float64)
    p = np.exp(l64 - l64.max(axis=1, keepdims=True))
    probs = p / p.sum(axis=1, keepdims=True)        # [NTOK, E]
    frac_probs = probs.mean(axis=0)
    frac_tokens = np.bincount(top_idx[:, 0], minlength=E) / NTOK
    aux = E * np.sum(frac_tokens * frac_probs) + np.mean(l64 ** 2) * 0.001
    aux_loss = np.float32(aux)

    # ---- dispatch: group tokens by expert ----
    flat_tok = np.repeat(np.arange(NTOK), TOPK)
    flat_e = top_idx.ravel()
    flat_w = weights.ravel()
    esort = np.argsort(flat_e, kind="stable")
    s_tok, s_e, s_w = flat_tok[esort], flat_e[esort], flat_w[esort]
    bounds = np.searchsorted(s_e, np.arange(E + 1))
    counts = np.diff(bounds)
    C, tiles = _plan_tiles(counts.max())

    tok_ids, tok_w, in_maps = [], [], []
    for e in range(E):
        ids = s_tok[bounds[e]:bounds[e + 1]]
        wts = s_w[bounds[e]:bounds[e + 1]]
        ce = len(ids)
        tok_ids.append(ids)
        tok_w.append(wts)
        xdt = ml_dtypes.bfloat16 if MODE == "bf16" else np.float32
        xTe = np.zeros((D, C), dtype=xdt)
        xTe[:, :ce] = x2d[ids].T.astype(xdt)
        w1e = np.ascontiguousarray(w1[e])
        w2e = np.ascontiguousarray(w2[e])
        if MODE in ("mixed", "bf16"):
            w2e = w2e.astype(ml_dtypes.bfloat16)
        if MODE == "bf16":
            w1e = w1e.astype(ml_dtypes.bfloat16)
        in_maps.append({
            "xT": xTe,
            "w1": w1e,
            "w2": w2e,
            "b1": np.ascontiguousarray(b1[e].reshape(MF, 128).T),
            "b2": np.ascontiguousarray(b2[e].reshape(MD, 128).T),
        })

    # ---- device: expert FFN on 8 cores ----
    nc = _build(C, tiles)
    # NTFF tracing needs antenv.axon_hooks, absent in this container —
    # make sure an inherited BASS_TRACE can't crash the run.
    os.environ["BASS_NEVER_TRACE"] = "1"
    res = run_bass_kernel_spmd(nc, in_maps, list(range(N_CORES)))
    kernel.last_results = res

    # ---- combine ----
    out2d = np.zeros((NTOK, D), dtype=np.float32)
    for e in range(E):
        ce = len(tok_ids[e])
        ye = res.results[e]["yT"][:, :ce]           # [D, ce]
        # token ids are unique within one expert (top-2 experts distinct)
        out2d[tok_ids[e]] += (tok_w[e][None, :] * ye).T
    out = out2d.reshape(B, T, D)
    return out, aux_loss


if __name__ == "__main__":
    data = np.load("/root/problem/inputs_cache.npz")
    inputs = {k: data[k] for k in data.files}
    ref = np.load("/root/problem/ref_cache.npz")
    out, aux = kernel(**inputs)
    ref_out, ref_aux = ref["out"], ref["aux"]
    denom = np.abs(ref_out).max()
    err = np.abs(out - ref_out).max()
    print(f"out absmax err: {err:.3e}  rel: {err/denom:.3e}")
    print(f"aux: {aux!r} vs ref {ref_aux!r}  diff {abs(float(aux)-float(ref_aux)):.3e}")
